# revision 15
# baseline (speedup 1.0000x reference)
"""LlamaAttention (B=1,S=2048,D=4096,NH=32,NKV=8,HD=128) on 8 TRN2 NeuronCores.

Sharding: tensor-parallel over heads (4 Q heads + 1 KV head per core).
Everything on-device runs in a transposed [feature, seq] layout so no PE
transposes are needed anywhere:
  - host ships x^T, wqkv^T-shard, wo^T-shard, cos^T/sin^T as bf16
  - QKV projection produces Q^T/K^T directly; V is produced in natural
    [seq, hd] layout (it is the AV matmul's stationary operand)
  - scores_T[k,q] = (K^T)^T . Q^T per 128x512 tile; exp on ACT engine
  - softmax denominator: exp tiles pair-summed on GpSimd, then a
    ones-vector matmul chain on PE reduces over partitions
  - y^T accumulated in PSUM, normalized with a partition-broadcast
    reciprocal (rank-1 ones outer product on PE)
  - AllGather is CHUNKED per 512-seq q-block and overlapped with the
    next q-block's attention and the wo projection of earlier q-blocks
  - out[qsl, e-shard] = y^T_block^T @ wo^T-shard per core, staggered one
    q-block behind the AllGathers; host concatenates 8 column shards
All bulk HBM traffic uses batched 3D-access-pattern DMAs (one descriptor
per multi-tile panel) to keep the DMA-trigger sequencers off the
critical path, spread across the SP/ACT/DVE trigger queues.
Mask handling is chosen host-side: causal fast path (skip upper-tri
k-tiles, additive diagonal patterns), all-zeros path (no mask at all), or
general path (stream mask^T/scale tiles and add before exp).
"""

import os
import sys
from contextlib import ExitStack

sys.path.insert(0, "/opt/trn_rl_repo")

import ml_dtypes
import numpy as np

import concourse.bass as bass
import concourse.mybir as mybir
import concourse.tile as tile
from concourse import bacc, bass_utils

F32 = mybir.dt.float32
BF16 = mybir.dt.bfloat16

B, S, D = 1, 2048, 4096
NH, NKV, HD = 32, 8, 128
NCORES = 8
QH = NH // NCORES            # 4 Q heads per core
EQK = QH * HD + HD           # 640 cols of wqkT per core (4 Q heads + 1 K head)
ESH = D // NCORES            # 512 output cols per core
SCALE = 1.0 / float(np.sqrt(HD))
NEG = -1e9

SB = 512                     # seq block (matmul free dim)
NSB = S // SB                # 4
NKT = S // 128               # 16 k tiles
NDC = D // 128               # 32 contraction chunks

LAST_RESULT = None           # BassKernelResults of the most recent run


def _bf16(a):
    return np.ascontiguousarray(a).astype(ml_dtypes.bfloat16)


def _build_program(mask_mode: str, reps: int = 1) -> bass.Bass:
    if mask_mode == "general":
        return _build_program_general()

    causal = mask_mode == "causal"
    nc = bacc.Bacc(target_bir_lowering=False, trn_type="TRN2")

    xT = nc.declare_dram_parameter("xT", [D, S], BF16, isOutput=False)
    wqkT = nc.declare_dram_parameter("wqkT", [D, EQK], BF16, isOutput=False)
    wvT = nc.declare_dram_parameter("wvT", [D, HD], BF16, isOutput=False)
    woT = nc.declare_dram_parameter("woT", [D, ESH], BF16, isOutput=False)
    cosT = nc.declare_dram_parameter("cosT", [HD, S], BF16, isOutput=False)
    sinT = nc.declare_dram_parameter("sinT", [HD, S], BF16, isOutput=False)
    if causal:
        diagp = nc.declare_dram_parameter("diagp", [128, 4 * SB], BF16, isOutput=False)
    out = nc.declare_dram_parameter("out", [S, ESH], F32, isOutput=True)

    with tile.TileContext(nc) as tc, ExitStack() as ctx:
        persist = ctx.enter_context(tc.tile_pool(name="persist", bufs=1))
        dram = ctx.enter_context(tc.tile_pool(name="dram", bufs=1, space="DRAM"))

        def make_ag_tiles(rep):
            sfx = f"_r{rep}" if rep else ""
            agin = [dram.tile([QH * HD, SB], BF16, name=f"agin{q}{sfx}",
                              tag=f"agin{q}{sfx}")
                    for q in range(NSB)]
            agout = [dram.tile([D, SB], BF16, name=f"agout{q}{sfx}",
                               tag=f"agout{q}{sfx}", addr_space="Shared")
                     for q in range(NSB)]
            return agin, agout

        agin, agout = make_ag_tiles(0)

        # ---- resident weights / tables (batched panel DMAs on SP) ------
        # first wqk dc-chunks and the first x panel arrive in small pieces
        # so the first QKV chain starts within a few microseconds
        wqk_big = persist.tile([128, NDC * EQK], BF16, name="wqk", tag="wqk")
        xpool = ctx.enter_context(tc.tile_pool(name="x", bufs=2))

        def load_x_half(sb, half, pieces=1):
            t = xpool.tile([128, 16 * SB], BF16, name="xh", tag="xh")
            for pc in range(pieces):
                w = 16 // pieces
                nc.sync.dma_start(
                    out=t[:, pc * w * SB:(pc + 1) * w * SB]
                        .rearrange("p (i c) -> p i c", i=w),
                    in_=xT[half * 2048 + pc * w * 128:
                           half * 2048 + (pc + 1) * w * 128,
                           sb * SB:(sb + 1) * SB]
                        .rearrange("(i p) c -> p i c", p=128))
            return t

        def load_wqk(g, n, pieces=1):
            for pc in range(pieces):
                w = n // pieces
                g0 = g + pc * w
                nc.sync.dma_start(
                    out=wqk_big[:, g0 * EQK:(g0 + w) * EQK]
                        .rearrange("p (i e) -> p i e", i=w),
                    in_=wqkT[g0 * 128:(g0 + w) * 128, :]
                        .rearrange("(i p) e -> p i e", p=128))

        load_wqk(0, 8, pieces=4)
        x00 = load_x_half(0, 0, pieces=4)
        load_wqk(8, 8)
        x01 = load_x_half(0, 1)
        load_wqk(16, 16)
        first_x = [x00, x01]

        wv_big = persist.tile([128, NDC * HD], BF16, name="wv", tag="wv")
        nc.sync.dma_start(
            out=wv_big[:].rearrange("p (i e) -> p i e", i=NDC),
            in_=wvT[:].rearrange("(i p) e -> p i e", p=128))
        cos_sb = persist.tile([HD, S], BF16, name="cos", tag="cos")
        nc.sync.dma_start(out=cos_sb[:], in_=cosT[:, :])
        sin_sb = persist.tile([HD, S], BF16, name="sin", tag="sin")
        nc.sync.dma_start(out=sin_sb[:], in_=sinT[:, :])
        if causal:
            diag_sb = persist.tile([128, 4 * SB], BF16, name="diag", tag="diag")
            nc.sync.dma_start(out=diag_sb[:], in_=diagp[:, :])
        wo_big = persist.tile([128, NDC * ESH], BF16, name="wo", tag="wo")
        for g in range(2):
            nc.sync.dma_start(
                out=wo_big[:, g * 16 * ESH:(g + 1) * 16 * ESH]
                    .rearrange("p (i e) -> p i e", i=16),
                in_=woT[g * 2048:(g + 1) * 2048, :]
                    .rearrange("(i p) e -> p i e", p=128))

        ones_bf = persist.tile([128, 1], BF16, name="ones_bf", tag="ones_bf")
        nc.vector.memset(ones_bf[:], 1.0)
        ones_row = persist.tile([1, 128], BF16, name="ones_row", tag="ones_row")
        nc.vector.memset(ones_row[:], 1.0)

        # persistent activations: Q^T per head + K^T (RoPE'd in place), V
        qkT = [persist.tile([HD, S], BF16, name=f"qkT{e}", tag=f"qkT{e}")
               for e in range(QH + 1)]
        v_sb = persist.tile([128, S], BF16, name="v", tag="v")

        qkvps = ctx.enter_context(tc.tile_pool(name="qkvps", bufs=2, space="PSUM"))
        spool = ctx.enter_context(tc.tile_pool(name="sc_ps", bufs=2, space="PSUM"))
        wops = ctx.enter_context(tc.tile_pool(name="wo_ps", bufs=1, space="PSUM"))
        ypool = ctx.enter_context(tc.tile_pool(name="y_ps", bufs=1, space="PSUM"))
        dpool = ctx.enter_context(tc.tile_pool(name="d_ps", bufs=1, space="PSUM"))
        rtmp = ctx.enter_context(tc.tile_pool(name="rtmp", bufs=2))
        epool = ctx.enter_context(tc.tile_pool(name="exp", bufs=6))
        ppool = ctx.enter_context(tc.tile_pool(name="pair", bufs=2))
        opool = ctx.enter_context(tc.tile_pool(name="attout", bufs=2))
        ybpool = ctx.enter_context(tc.tile_pool(name="yb", bufs=2))
        osb = ctx.enter_context(tc.tile_pool(name="osb", bufs=2))

        H2 = HD // 2

        def qkv_block(sb, xh):
            # Q^T / K^T: 5 chains of 32 matmuls, FD=512
            for et in range(QH + 1):
                ps = qkvps.tile([128, SB], F32, name="ps", tag="ps")
                for dc in range(NDC):
                    nc.tensor.matmul(
                        ps[:],
                        lhsT=wqk_big[:, dc * EQK + et * 128:dc * EQK + (et + 1) * 128],
                        rhs=xh[dc // 16][:, (dc % 16) * SB:(dc % 16 + 1) * SB],
                        start=(dc == 0), stop=(dc == NDC - 1))
                nc.scalar.copy(qkT[et][:, sb * SB:(sb + 1) * SB], ps[:])
            # V natural: 4 chains of 32 matmuls (FD=128) into free-dim
            # slices of one shared psum tile, drained with a single copy
            psv = qkvps.tile([128, SB], F32, name="ps", tag="ps")
            for st in range(SB // 128):
                for dc in range(NDC):
                    nc.tensor.matmul(
                        psv[:, st * 128:(st + 1) * 128],
                        lhsT=xh[dc // 16][:, (dc % 16) * SB + st * 128:
                                          (dc % 16) * SB + (st + 1) * 128],
                        rhs=wv_big[:, dc * HD:(dc + 1) * HD],
                        start=(dc == 0), stop=(dc == NDC - 1))
            nc.scalar.copy(v_sb[:, sb * SB:(sb + 1) * SB], psv[:])
            # RoPE in place on the 5 fresh [HD, SB] slices (DVE)
            sl = slice(sb * SB, (sb + 1) * SB)
            for et in range(QH + 1):
                src = qkT[et]
                rot = rtmp.tile([128, SB], BF16, name="rot", tag="rot")
                nc.vector.tensor_copy(rot[0:H2, :], src[H2:HD, sl])
                nc.vector.tensor_copy(rot[H2:HD, :], src[0:H2, sl])
                t1 = rtmp.tile([128, SB], BF16, name="t1", tag="t1")
                nc.vector.tensor_tensor(
                    t1[:], src[:, sl], cos_sb[:, sl], mybir.AluOpType.mult)
                t2 = rtmp.tile([128, SB], BF16, name="t2", tag="t2")
                nc.vector.tensor_tensor(
                    t2[:], rot[:], sin_sb[:, sl], mybir.AluOpType.mult)
                nc.vector.tensor_tensor(
                    src[:, sl], t1[:], t2[:], mybir.AluOpType.add)

        def attention_block(qb):
            qsl = slice(qb * SB, (qb + 1) * SB)
            klim = (qb + 1) * (SB // 128) if causal else NKT
            for h in range(QH):
                ps_y = ypool.tile([HD, SB], F32, name="psy", tag="psy")
                ps_d = dpool.tile([1, SB], F32, name="psd", tag="psd")
                et_prev = None
                for kt in range(klim):
                    ps_s = spool.tile([128, SB], F32, name="pss", tag="pss")
                    nc.tensor.matmul(
                        ps_s[:],
                        lhsT=qkT[QH][:, kt * 128:(kt + 1) * 128],
                        rhs=qkT[h][:, qsl],
                        start=True, stop=True)
                    et = epool.tile([128, SB], BF16, name="et", tag="et")
                    if causal and kt >= qb * (SB // 128):
                        j = kt - qb * (SB // 128)
                        nc.vector.tensor_tensor(
                            ps_s[:], ps_s[:],
                            diag_sb[:, j * SB:(j + 1) * SB],
                            mybir.AluOpType.add)
                    nc.scalar.activation(
                        et[:], ps_s[:],
                        mybir.ActivationFunctionType.Exp, scale=SCALE)
                    nc.tensor.matmul(
                        ps_y[:],
                        lhsT=v_sb[:, kt * 128:(kt + 1) * 128],
                        rhs=et[:],
                        start=(kt == 0), stop=(kt == klim - 1))
                    # denominator: pair-sum on GpSimd, reduce pairs on PE
                    if kt % 2 == 0:
                        et_prev = et
                    else:
                        pr = ppool.tile([128, SB], BF16, name="pr", tag="pr")
                        nc.gpsimd.tensor_tensor(
                            pr[:], et_prev[:], et[:], mybir.AluOpType.add)
                        nc.tensor.matmul(
                            ps_d[:], lhsT=ones_bf[:], rhs=pr[:],
                            start=(kt == 1), stop=(kt == klim - 1))
                recip = opool.tile([1, SB], BF16, name="recip", tag="recip")
                with nc.allow_low_precision(
                        reason="softmax denom is positive and O(100); bf16 "
                               "reciprocal feeds a bf16 broadcast anyway"):
                    nc.vector.reciprocal(recip[:], ps_d[:])
                # broadcast along partitions via rank-1 outer product
                ps_r = spool.tile([HD, SB], F32, name="psr", tag="psr", bufs=1)
                nc.tensor.matmul(
                    ps_r[:], lhsT=ones_row[:], rhs=recip[:],
                    start=True, stop=True)
                rb = opool.tile([HD, SB], F32, name="rb", tag="rb")
                nc.scalar.copy(rb[:], ps_r[:])
                ynorm = opool.tile([HD, SB], BF16, name="ynorm", tag="ynorm")
                nc.vector.tensor_tensor(
                    ynorm[:], ps_y[:], rb[:], mybir.AluOpType.mult)
                nc.sync.dma_start(
                    out=agin[qb][h * HD:(h + 1) * HD, :], in_=ynorm[:])

        def allgather_block(qb):
            nc.gpsimd.collective_compute(
                "AllGather",
                mybir.AluOpType.bypass,
                replica_groups=[list(range(NCORES))],
                ins=[agin[qb][:].opt()],
                outs=[agout[qb][:].opt()],
            )

        def wo_block(qb, rep=0):
            # load y^T panel for this q-block (2 halves, batched DMAs).
            # tile_wait_until keeps the scheduler from hoisting these
            # triggers (which wait on the qb-th AllGather) ahead of younger
            # compute on the same engine queue — the schedule-time model
            # treats collectives as near-instant.
            yb = []
            with tc.tile_wait_until(0.46 * rep + 0.20 + 0.11 * qb):
                for half in range(2):
                    t = ybpool.tile([128, 16 * SB], BF16, name="ybt", tag="ybt")
                    nc.scalar.dma_start(
                        out=t[:].rearrange("p (i c) -> p i c", i=16),
                        in_=agout[qb][half * 2048:(half + 1) * 2048, :]
                            .rearrange("(i p) c -> p i c", p=128))
                    yb.append(t)
            # out[qsl, :] natural; 4 sequential s-tile chains (1 PSUM bank,
            # own pool so the shared qkv/V ring never waits on a collective)
            for st in range(4):
                pso = wops.tile([128, ESH], F32, name="pso", tag="pso")
                for dc in range(NDC):
                    nc.tensor.matmul(
                        pso[:],
                        lhsT=yb[dc // 16][:, (dc % 16) * SB + st * 128:
                                          (dc % 16) * SB + (st + 1) * 128],
                        rhs=wo_big[:, dc * ESH:(dc + 1) * ESH],
                        start=(dc == 0), stop=(dc == NDC - 1))
                ot = osb.tile([128, ESH], F32, name="ot", tag="ot")
                nc.vector.tensor_copy(ot[:], pso[:])
                nc.sync.dma_start(
                    out=out[qb * SB + st * 128:qb * SB + (st + 1) * 128, :],
                    in_=ot[:])

        # ---- emission schedule ---------------------------------------
        # causal: QKV(sb) then attention(qb=sb) interleave; AllGather per
        # q-block fires as soon as its attention is done; wo stays one
        # q-block behind the AllGathers so PE never stalls on a collective.
        # reps>1 repeats the whole body (steady-state timing harness).
        for rep in range(reps):
            if rep == 0:
                xh = first_x
            else:
                # Shared DRAM collective outputs allow a single writer each:
                # fresh scratch per rep (timing-harness builds only)
                agin, agout = make_ag_tiles(rep)
                xh = [load_x_half(0, 0), load_x_half(0, 1)]
            if causal:
                for sb in range(NSB):
                    qkv_block(sb, xh)
                    if sb + 1 < NSB:
                        nxt0 = load_x_half(sb + 1, 0)
                        nxt1 = load_x_half(sb + 1, 1)
                    attention_block(sb)
                    allgather_block(sb)
                    if sb >= 1:
                        wo_block(sb - 1, rep)
                    if sb + 1 < NSB:
                        xh = [nxt0, nxt1]
                wo_block(NSB - 1, rep)
            else:
                for sb in range(NSB):
                    qkv_block(sb, xh)
                    if sb + 1 < NSB:
                        nxt0 = load_x_half(sb + 1, 0)
                        nxt1 = load_x_half(sb + 1, 1)
                        xh = [nxt0, nxt1]
                for qb in range(NSB):
                    attention_block(qb)
                    allgather_block(qb)
                    if qb >= 1:
                        wo_block(qb - 1, rep)
                wo_block(NSB - 1, rep)

    nc.finalize()
    return nc


def _build_program_general() -> bass.Bass:
    """Fallback for arbitrary (non-causal, non-zero) masks: the original
    unchunked pipeline with the mask streamed and added before exp."""
    nc = bacc.Bacc(target_bir_lowering=False, trn_type="TRN2")

    xT = nc.declare_dram_parameter("xT", [D, S], BF16, isOutput=False)
    wqkT = nc.declare_dram_parameter("wqkT", [D, EQK], BF16, isOutput=False)
    wvT = nc.declare_dram_parameter("wvT", [D, HD], BF16, isOutput=False)
    woT = nc.declare_dram_parameter("woT", [D, ESH], BF16, isOutput=False)
    cosT = nc.declare_dram_parameter("cosT", [HD, S], BF16, isOutput=False)
    sinT = nc.declare_dram_parameter("sinT", [HD, S], BF16, isOutput=False)
    maskT = nc.declare_dram_parameter("maskT", [S, S], F32, isOutput=False)
    out = nc.declare_dram_parameter("out", [S, ESH], F32, isOutput=True)

    with tile.TileContext(nc) as tc, ExitStack() as ctx:
        persist = ctx.enter_context(tc.tile_pool(name="persist", bufs=1))
        dram = ctx.enter_context(tc.tile_pool(name="dram", bufs=1, space="DRAM"))

        ag_in = dram.tile([QH * HD, S], BF16, name="ag_in", tag="ag_in")
        ag_out = dram.tile([D, S], BF16, name="ag_out", tag="ag_out",
                           addr_space="Shared")

        wqk_sb = []
        for dc in range(NDC):
            t = persist.tile([128, EQK], BF16, name=f"wqk{dc}", tag=f"wqk{dc}")
            nc.sync.dma_start(out=t[:], in_=wqkT[dc * 128:(dc + 1) * 128, :])
            wqk_sb.append(t)
        wv_sb = []
        for dc in range(NDC):
            t = persist.tile([128, HD], BF16, name=f"wv{dc}", tag=f"wv{dc}")
            nc.sync.dma_start(out=t[:], in_=wvT[dc * 128:(dc + 1) * 128, :])
            wv_sb.append(t)
        cos_sb = persist.tile([HD, S], BF16, name="cos", tag="cos")
        nc.sync.dma_start(out=cos_sb[:], in_=cosT[:, :])
        sin_sb = persist.tile([HD, S], BF16, name="sin", tag="sin")
        nc.sync.dma_start(out=sin_sb[:], in_=sinT[:, :])
        ones_sb = persist.tile([128, 1], BF16, name="ones", tag="ones")
        nc.vector.memset(ones_sb[:], 1.0)
        ones_row = persist.tile([1, 128], F32, name="ones_row", tag="ones_row")
        nc.vector.memset(ones_row[:], 1.0)
        ones_f32 = persist.tile([128, 1], F32, name="ones_f32", tag="ones_f32")
        nc.vector.memset(ones_f32[:], 1.0)

        qkT_sb = [persist.tile([HD, S], BF16, name=f"qkT{e}", tag=f"qkT{e}") for e in range(QH + 1)]
        ropT_sb = [persist.tile([HD, S], BF16, name=f"ropT{e}", tag=f"ropT{e}") for e in range(QH + 1)]
        v_sb = persist.tile([128, S], BF16, name="v", tag="v")

        with tc.tile_pool(name="xT", bufs=2 * NDC + 4) as xpool, \
             tc.tile_pool(name="qkvps", bufs=2, space="PSUM") as qkvps, \
             tc.tile_pool(name="ropetmp", bufs=4) as rtmp:
            for sb in range(NSB):
                xts = []
                for dc in range(NDC):
                    t = xpool.tile([128, SB], BF16, name="xt", tag="xt")
                    nc.sync.dma_start(
                        out=t[:], in_=xT[dc * 128:(dc + 1) * 128, sb * SB:(sb + 1) * SB])
                    xts.append(t)
                for et in range(QH + 1):
                    ps = qkvps.tile([128, SB], F32, name="ps", tag="ps")
                    for dc in range(NDC):
                        nc.tensor.matmul(
                            ps[:],
                            lhsT=wqk_sb[dc][:, et * 128:(et + 1) * 128],
                            rhs=xts[dc][:],
                            start=(dc == 0), stop=(dc == NDC - 1))
                    nc.scalar.copy(qkT_sb[et][:, sb * SB:(sb + 1) * SB], ps[:])
                for st in range(SB // 128):
                    ps = qkvps.tile([128, HD], F32, name="psv", tag="psv")
                    for dc in range(NDC):
                        nc.tensor.matmul(
                            ps[:],
                            lhsT=xts[dc][:, st * 128:(st + 1) * 128],
                            rhs=wv_sb[dc][:],
                            start=(dc == 0), stop=(dc == NDC - 1))
                    s0 = sb * SB + st * 128
                    nc.scalar.copy(v_sb[:, s0:s0 + 128], ps[:])

            H2 = HD // 2
            for e in range(QH + 1):
                for sb in range(NSB):
                    sl = slice(sb * SB, (sb + 1) * SB)
                    src = qkT_sb[e]
                    rot = rtmp.tile([128, SB], BF16, name="rot", tag="rot")
                    nc.vector.tensor_copy(rot[0:H2, :], src[H2:HD, sl])
                    nc.vector.tensor_copy(rot[H2:HD, :], src[0:H2, sl])
                    t1 = rtmp.tile([128, SB], BF16, name="t1", tag="t1")
                    nc.vector.tensor_tensor(
                        t1[:], src[:, sl], cos_sb[:, sl], mybir.AluOpType.mult)
                    t2 = rtmp.tile([128, SB], BF16, name="t2", tag="t2")
                    nc.vector.tensor_tensor(
                        t2[:], rot[:], sin_sb[:, sl], mybir.AluOpType.mult)
                    nc.vector.tensor_tensor(
                        ropT_sb[e][:, sl], t1[:], t2[:], mybir.AluOpType.add)

        kT = ropT_sb[QH]
        with ExitStack() as actx:
            mpool = actx.enter_context(tc.tile_pool(name="mask", bufs=NKT + 2))
            spool = actx.enter_context(tc.tile_pool(name="sc_ps", bufs=3, space="PSUM"))
            ypool = actx.enter_context(tc.tile_pool(name="y_ps", bufs=2, space="PSUM"))
            dpool = actx.enter_context(tc.tile_pool(name="d_ps", bufs=2, space="PSUM"))
            epool = actx.enter_context(tc.tile_pool(name="exp", bufs=6))
            opool = actx.enter_context(tc.tile_pool(name="attout", bufs=4))

            for qb in range(NSB):
                qsl = slice(qb * SB, (qb + 1) * SB)
                klim = NKT
                mtiles = []
                for kt in range(klim):
                    mt = mpool.tile([128, SB], F32, name="mt", tag="mt")
                    nc.sync.dma_start(
                        out=mt[:],
                        in_=maskT[kt * 128:(kt + 1) * 128, qsl])
                    mtiles.append(mt)
                for h in range(QH):
                    ps_y = ypool.tile([HD, SB], F32, name="psy", tag="psy")
                    ps_d = dpool.tile([1, SB], F32, name="psd", tag="psd")
                    dsum = opool.tile([128, SB], F32, name="dsum", tag="dsum")
                    for kt in range(klim):
                        ps_s = spool.tile([128, SB], F32, name="pss", tag="pss")
                        nc.tensor.matmul(
                            ps_s[:],
                            lhsT=kT[:, kt * 128:(kt + 1) * 128],
                            rhs=ropT_sb[h][:, qsl],
                            start=True, stop=True)
                        et = epool.tile([128, SB], BF16, name="et", tag="et")
                        nc.vector.tensor_tensor(
                            ps_s[:], ps_s[:], mtiles[kt][:],
                            mybir.AluOpType.add)
                        nc.scalar.activation(
                            et[:], ps_s[:],
                            mybir.ActivationFunctionType.Exp, scale=SCALE)
                        nc.tensor.matmul(
                            ps_y[:],
                            lhsT=v_sb[:, kt * 128:(kt + 1) * 128],
                            rhs=et[:],
                            start=(kt == 0), stop=(kt == klim - 1))
                        if kt == 0:
                            nc.vector.tensor_copy(dsum[:], et[:])
                        else:
                            nc.vector.tensor_tensor(
                                dsum[:], dsum[:], et[:], mybir.AluOpType.add)
                    nc.tensor.matmul(
                        ps_d[:], lhsT=ones_f32[:], rhs=dsum[:],
                        start=True, stop=True)
                    recip = opool.tile([1, SB], F32, name="recip", tag="recip")
                    nc.vector.reciprocal(recip[:], ps_d[:])
                    ps_r = dpool.tile([HD, SB], F32, name="psr", tag="psr", bufs=1)
                    nc.tensor.matmul(
                        ps_r[:], lhsT=ones_row[:], rhs=recip[:],
                        start=True, stop=True)
                    rb = opool.tile([HD, SB], F32, name="rb", tag="rb")
                    nc.scalar.copy(rb[:], ps_r[:])
                    ynorm = opool.tile([HD, SB], BF16, name="ynorm", tag="ynorm")
                    nc.vector.tensor_tensor(
                        ynorm[:], ps_y[:], rb[:], mybir.AluOpType.mult)
                    nc.sync.dma_start(
                        out=ag_in[h * HD:(h + 1) * HD, qsl], in_=ynorm[:])

        nc.gpsimd.collective_compute(
            "AllGather",
            mybir.AluOpType.bypass,
            replica_groups=[list(range(NCORES))],
            ins=[ag_in[:].opt()],
            outs=[ag_out[:].opt()],
        )

        with tc.tile_pool(name="wo", bufs=1) as wpool, \
             tc.tile_pool(name="yt", bufs=NDC + 8) as ytpool, \
             tc.tile_pool(name="ops", bufs=2, space="PSUM") as opsp, \
             tc.tile_pool(name="osb", bufs=4) as osbp:
            wo_sb = []
            for dc in range(NDC):
                t = wpool.tile([128, ESH], BF16, name=f"wo{dc}", tag=f"wo{dc}")
                nc.sync.dma_start(out=t[:], in_=woT[dc * 128:(dc + 1) * 128, :])
                wo_sb.append(t)
            for sg in range(NSB):
                yts = []
                for dc in range(NDC):
                    t = ytpool.tile([128, SB], BF16, name="yt", tag="yt")
                    nc.sync.dma_start(
                        out=t[:],
                        in_=ag_out[dc * 128:(dc + 1) * 128, sg * SB:(sg + 1) * SB])
                    yts.append(t)
                for stl in range(SB // 128):
                    ps = opsp.tile([128, ESH], F32, name="ps", tag="ps")
                    for dc in range(NDC):
                        nc.tensor.matmul(
                            ps[:],
                            lhsT=yts[dc][:, stl * 128:(stl + 1) * 128],
                            rhs=wo_sb[dc][:],
                            start=(dc == 0), stop=(dc == NDC - 1))
                    ot = osbp.tile([128, ESH], F32, name="ot", tag="ot")
                    nc.scalar.copy(ot[:], ps[:])
                    st = sg * (SB // 128) + stl
                    nc.sync.dma_start(
                        out=out[st * 128:(st + 1) * 128, :], in_=ot[:])

    nc.finalize()
    return nc


_PROG_CACHE = {}


def _mask_mode_and_aux(mask):
    m = np.asarray(mask).reshape(S, S)
    if not m.any():
        return "zeros", None
    tril = np.tril(np.ones((S, S), dtype=bool))
    if (m[tril] == 0.0).all() and (m[~tril] == NEG).all():
        return "causal", None
    return "general", np.ascontiguousarray(m.T / SCALE).astype(np.float32)


def _prepare(x, mask, wqkv, wo):
    x = np.asarray(x, dtype=np.float32)
    wqkv = np.asarray(wqkv, dtype=np.float32)
    wo = np.asarray(wo, dtype=np.float32)

    mode, maskT = _mask_mode_and_aux(mask)

    xT = _bf16(x.reshape(S, D).T)                       # [D, S]
    inv = 1.0 / (10000.0 ** (np.arange(0, HD, 2, dtype=np.float32) / HD))
    t = np.arange(S, dtype=np.float32)
    freqs = np.outer(t, inv)                            # [S, HD/2]
    emb = np.concatenate([freqs, freqs], axis=-1)       # [S, HD]
    cosT = _bf16(np.cos(emb).T)                         # [HD, S]
    sinT_np = np.sin(emb).T.copy()                      # [HD, S]
    sinT_np[:HD // 2] *= -1.0                           # bake rotate_half sign
    sinT = _bf16(sinT_np)

    if mode == "causal":
        # additive pattern for diagonal tile j (k0 = qb*512 + j*128):
        # allow when q >= k, i.e. qq >= j*128 + kk  (qq, kk within tile)
        kk = np.arange(128)[:, None]
        qq = np.arange(SB)[None, :]
        pats = []
        for j in range(4):
            allow = qq >= (j * 128 + kk)
            pats.append(np.where(allow, 0.0, NEG / SCALE).astype(np.float32))
        diagp = _bf16(np.concatenate(pats, axis=1))     # [128, 2048] bf16

    in_maps = []
    for r in range(NCORES):
        q_rows = wqkv[r * QH * HD:(r + 1) * QH * HD]            # [512, D]
        k_rows = wqkv[NH * HD + r * HD: NH * HD + (r + 1) * HD]  # [128, D]
        v_rows = wqkv[(NH + NKV) * HD + r * HD:(NH + NKV) * HD + (r + 1) * HD]
        im = {
            "xT": xT,
            "wqkT": _bf16(np.concatenate([q_rows, k_rows], axis=0).T),  # [D, 640]
            "wvT": _bf16(v_rows.T),                                     # [D, 128]
            "woT": _bf16(wo[r * ESH:(r + 1) * ESH, :].T),               # [D, 512]
            "cosT": cosT,
            "sinT": sinT,
        }
        if mode == "general":
            im["maskT"] = maskT
        elif mode == "causal":
            im["diagp"] = diagp
        in_maps.append(im)
    return mode, in_maps


def kernel(x, mask, wqkv, wo):
    global LAST_RESULT
    mode, in_maps = _prepare(x, mask, wqkv, wo)

    if mode not in _PROG_CACHE:
        _PROG_CACHE[mode] = _build_program(mode)
    nc = _PROG_CACHE[mode]

    res = bass_utils.run_bass_kernel_spmd(
        nc, in_maps, core_ids=list(range(NCORES)),
        trace=bool(os.environ.get("BASS_TRACE")),
    )
    LAST_RESULT = res

    shards = [np.asarray(res.results[r]["out"], dtype=np.float32)
              for r in range(NCORES)]
    full = np.concatenate(shards, axis=1)               # [S, D]
    return full.reshape(B, S, D)
